# revision 16
# baseline (speedup 1.0000x reference)
"""Trainium2 Bass kernel for single-head attention (B=8, S=2048, D=U=512).

Sharding: data-parallel over batch - one batch element per NeuronCore (8 cores).

Math: score = X W1 (V W2)^T / sqrt(U) = X M V^T with M = W1 W2^T folded once
per core (saves one full projection). context = softmax(score) (V W3).

Dataflow per core:
  Phase 1 (streamed with the serial DMA queue: V0a,V0b,W3,V1,W1,V2,W2,V3,X0):
    - V chunks PE-transposed (f32r) and quantized to fp8e4 hi/lo (vT8).
    - W1,W2 PE-transposed; M = W1 W2^T on PE, scaled by 16, fp8 hi/lo (m12_8).
      W3 scaled by 16, fp8 hi/lo (w3_8).
    - vN[j,u] = 16*(V W3) via fp8 DoubleRow (3-term hi/lo cross products),
      f32r, with column 512 = 16.0 (fused softmax denominator).
    - X group 0 transposed/quantized (xT8); qmT(0) = 16*(M^T x^T) via
      DoubleRow, quantized hi/lo (qmT8).
  Phase 2 per i-block ib (pipelined):
    scores(ib): scoresT[j,i] = sum_e vT[e,j] qmT[e,i] via fp8 DoubleRow;
      exp on ACT -> expB (f32r). X group ib+1 transpose + qmT(ib+1) are
      interleaved here (PE) so their vector ops land in phase-2 ACT/DVE slack.
    ctx(ib-1): ctx[i,u] (+den col) = sum_j expB[j,i] vN[j,u], f32r matmuls in
      two psum groups (258+256 cols); out = ctx * recip(den) -> DMA.

fp8 DoubleRow (operands paired along a leading free dim of 2) runs at 0.5
cycles/output-col with 256-deep contraction per call - 4x f32r throughput;
the 3-call hi/lo scheme nets 2.67x at ~8-bit per-term accuracy. Measured
end-to-end rel err ~3e-3 (threshold 2e-2).
"""

import math
import os
import sys

for _p in ("/opt/trn_rl_repo", os.path.expanduser("~/.axon_site/_ro/trn_rl_repo")):
    if os.path.isdir(_p) and _p not in sys.path:
        sys.path.insert(0, _p)

import numpy as np

import concourse.bass as bass
import concourse.tile as tile
from concourse import bacc, mybir
from concourse.bass import ts
from concourse.bass_utils import run_bass_kernel_spmd
from concourse.masks import make_identity

F32 = mybir.dt.float32
F32R = mybir.dt.float32r
F8 = mybir.dt.float8e4
EXP = mybir.ActivationFunctionType.Exp
DR = mybir.MatmulPerfMode.DoubleRow
MUL = mybir.AluOpType.mult
SUB = mybir.AluOpType.subtract

P = 128          # partitions
B = 8            # batch (one element per core)
S = 2048         # sequence length
D = 512          # model dim
U = 512          # units
DC = D // P      # 4 chunks of the contraction dims
SC = S // P      # 16 s-chunks
IB = 512         # i-block (query positions per attention block)
NIB = S // IB    # 4
ICC = IB // P    # 4 i-chunks per block
SCALE = 1.0 / math.sqrt(float(U))
WSC = 16.0       # weight pre-scale so fp8 quantization stays in normal range
VNF = 520        # vN free width: 512 u-cols + col 512 = WSC (den) + pad
CA = 258         # ctx psum group A columns (u 0..257)
CB = 256         # ctx psum group B columns (u 258..511, den at 254, pad)

# DoubleRow hi/lo call list: (stationary half, moving half)
HL3 = ((0, 0), (0, 1), (1, 0))


def _emit(nc, tc, q_d, v_d, w1_d, w2_d, w3_d, o_d):
    with tc.tile_pool(name="const", bufs=1) as cp, \
         tc.tile_pool(name="persist", bufs=1) as pp:
        identf = cp.tile([P, P], F32, name="identf")
        make_identity(nc, identf)
        ident = cp.tile([P, P], F32R, name="ident")
        nc.vector.tensor_copy(ident, identf)

        m12_8 = pp.tile([P, 2, DC, D], F8, name="m12_8")
        w3_8 = pp.tile([P, 2, DC, U], F8, name="w3_8")
        vT8 = pp.tile([P, 2, DC, S], F8, name="vT8")
        xT8 = pp.tile([P, 2, DC, S], F8, name="xT8")
        qmT8 = pp.tile([P, 2, DC, S], F8, name="qmT8")
        vN = pp.tile([P, 2, SC, VNF], F8, name="vN")
        # den column = WSC in the hi plane (vN holds 16*v so the num/den
        # scales cancel); zero pad cols and the lo plane's den region.
        nc.gpsimd.memset(vN[:, 0, :, U:VNF], 0.0)
        nc.gpsimd.memset(vN[:, 1, :, U:VNF], 0.0)
        nc.gpsimd.memset(vN[:, 0, :, U:U + 1], WSC)

        _veng = [0]

        def hilo(dst8, hl_idx, src, scale=1.0, dve_hi=False):
            # dst8[...,0,...] = fp8(scale*src) on ACT (or DVE if dve_hi);
            # dst8[...,1,...] = fp8(scale*src - hi) on DVE.
            hi = dst8[tuple([slice(None), 0] + hl_idx)]
            lo = dst8[tuple([slice(None), 1] + hl_idx)]
            if dve_hi:
                nc.vector.tensor_copy(hi, src)
            elif scale == 1.0:
                nc.scalar.copy(hi, src)
            else:
                nc.scalar.mul(hi, src, scale)
            nc.vector.scalar_tensor_tensor(lo, src, scale, hi, op0=MUL, op1=SUB)

        def copy_f32r(dst, src):
            _veng[0] += 1
            if _veng[0] % 2:
                nc.scalar.copy(dst, src)
            else:
                nc.vector.tensor_copy(dst, src)

        def dr6(ps, stat8, s_idx, mov8, m_idx):
            # 6 DoubleRow calls: 2 pair-chunks x (hh, hl, lh)
            n = 0
            for pc in (0, 2):
                for sh, mh in HL3:
                    n += 1
                    nc.tensor.matmul(
                        ps,
                        stat8[tuple([slice(None), sh, slice(pc, pc + 2)] + s_idx)],
                        mov8[tuple([slice(None), mh, slice(pc, pc + 2)] + m_idx)],
                        start=(n == 1), stop=(n == 6), perf_mode=DR)

        _phase1(nc, tc, q_d, v_d, w1_d, w2_d, w3_d, ident,
                m12_8, w3_8, vT8, xT8, qmT8, vN, hilo, copy_f32r, dr6)
        _phase2(nc, tc, q_d, o_d, ident, m12_8, vT8, xT8, qmT8, vN, hilo, dr6,
                copy_f32r, w3_8)


def _phase1(nc, tc, q_d, v_d, w1_d, w2_d, w3_d, ident,
            m12_8, w3_8, vT8, xT8, qmT8, vN, hilo, copy_f32r, dr6):
    with tc.tile_pool(name="wtmp", bufs=1) as wp, \
         tc.tile_pool(name="loadp", bufs=3) as loadp, \
         tc.tile_pool(name="tps", bufs=2, space="PSUM") as tpsp, \
         tc.tile_pool(name="pjps", bufs=2, space="PSUM") as pjps:
        w1n = wp.tile([P, DC, U], F32R, name="w1n")
        w2n = wp.tile([P, DC, U], F32R, name="w2n")
        w3n = wp.tile([P, DC, U], F32R, name="w3n")
        w1t = wp.tile([P, DC, D], F32R, name="w1t")
        w2t = wp.tile([P, DC, D], F32R, name="w2t")

        def transpose_pair(nat0, nat1, dst8, jc):
            # two 128-row chunks -> one 2-bank psum tile (layout [c, g, q] so
            # (g q) merges to a contiguous 256-wide dim) -> one hi + one lo op
            tp = tpsp.tile([P, DC, 2, P], F32R, tag="tp")
            for g, nat in ((0, nat0), (1, nat1)):
                for dc in range(DC):
                    nc.tensor.transpose(tp[:, dc, g, :], nat[:, ts(dc, P)], ident)
            src = tp.rearrange("p c g q -> p c (g q)")
            hi = dst8[:, 0, :, jc * P:(jc + 2) * P]
            lo = dst8[:, 1, :, jc * P:(jc + 2) * P]
            nc.scalar.copy(hi, src)
            nc.vector.scalar_tensor_tensor(lo, src, 1.0, hi, op0=MUL, op1=SUB)

        def emit_wt(wn, wt):
            # wt[u%128, uc, d] = W[d, u] transposed blocks
            for ucp in (0, 2):
                tp = tpsp.tile([P, DC, 2, P], F32R, tag="tp")
                for g in (0, 1):
                    for dc in range(DC):
                        nc.tensor.transpose(tp[:, dc, g, :],
                                            wn[:, dc, ts(ucp + g, P)], ident)
                    copy_f32r(
                        wt[:, ucp + g, :].rearrange("p (k q) -> p k q", k=DC),
                        tp[:, :, g, :])

        def emit_m12(dcp):
            # M[d,e] = sum_u W1[d,u] W2[e,u], row-chunks (2dcp, 2dcp+1)
            ps = pjps.tile([P, 2, D], F32, tag="pj")
            for g in (0, 1):
                for uc in range(DC):
                    nc.tensor.matmul(ps[:, g, :], w1t[:, uc, ts(2 * dcp + g, P)],
                                     w2t[:, uc, :],
                                     start=(uc == 0), stop=(uc == DC - 1))
                hilo(m12_8, [2 * dcp + g, slice(None)], ps[:, g, :], scale=WSC)

        def emit_vn2(jc):
            # vN[j,u] = 16 * sum_d V[j,d] W3[d,u], two j-chunks at once
            ps = pjps.tile([P, 2, U], F32, tag="pj")
            dr6(ps[:, 0, :], vT8, [ts(jc, P)], w3_8, [slice(None)])
            dr6(ps[:, 1, :], vT8, [ts(jc + 1, P)], w3_8, [slice(None)])
            hilo(vN, [slice(jc, jc + 2), slice(0, U)], ps)

        def emit_xt(sc, nat):
            tp = tpsp.tile([P, DC, 2, P], F32R, tag="tp")
            for dc in range(DC):
                nc.tensor.transpose(tp[:, dc, 0, :], nat[:, ts(dc, P)], ident)
            hilo(xT8, [slice(None), ts(sc, P)], tp[:, :, 0, :])

        def emit_qmt(ib):
            # qmT[e,i] = 16 * sum_d M[d,e] xT[d,i] for i-block ib
            for ecp in (0, 2):
                ps = pjps.tile([P, 2, IB], F32, tag="pj")
                for g in (0, 1):
                    dr6(ps[:, g, :], m12_8, [ts(ecp + g, P)], xT8, [ts(ib, IB)])
                    hilo(qmT8, [ecp + g, ts(ib, IB)], ps[:, g, :])

        # DMA queue: V0a(chunk0), W3, V0b(1-3), V1, W1, V2, W2, V3, X0.
        # W3 right after the first chunk so vN matmuls can start early; the
        # vN stream then fills every later DMA-arrival stall.
        nat_v = [loadp.tile([P, 4, D], F32R, tag="nat", name=f"nat_v{g}")
                 for g in range(2)]
        for c in range(4):
            nc.sync.dma_start(
                nat_v[0][:, c:c + 1, :],
                v_d[ts(c, P), :].rearrange("(c p) d -> p c d", p=P))
        nc.sync.dma_start(w3n, w3_d.rearrange("(c p) u -> p c u", p=P))
        nc.sync.dma_start(nat_v[1],
                          v_d[ts(1, 4 * P), :].rearrange("(c p) d -> p c d", p=P))
        nc.sync.dma_start(w1n, w1_d.rearrange("(c p) u -> p c u", p=P))

        def tp_single(jc, nat):
            tp = tpsp.tile([P, DC, 2, P], F32R, tag="tp")
            for dc in range(DC):
                nc.tensor.transpose(tp[:, dc, 0, :], nat[:, ts(dc, P)], ident)
            hilo(vT8, [slice(None), ts(jc, P)], tp[:, :, 0, :])

        for c in range(4):
            tp_single(c, nat_v[0][:, c, :])
        hilo(w3_8, [slice(None), slice(None)], w3n, scale=WSC)
        nat_v.append(None)
        nat_v.append(None)
        sched = [
            ("vn", 0), ("vn", 2),
            ("dma_v", 2), ("dma_w2", None),
            ("tp", 4), ("vn", 4), ("tp", 6),
            ("wt", 1), ("vn", 6),
            ("dma_v", 3),
            ("tp", 8), ("vn", 8), ("tp", 10),
            ("wt", 2), ("vn", 10),
            ("dma_x", 0),
            ("m12", 0), ("tp", 12), ("m12", 1), ("tp", 14),
            ("xt", 0), ("xt", 1), ("xt", 2), ("xt", 3),
            ("qmt", 0),
        ]
        nat_x = None
        for op, arg in sched:
            if op == "dma_v":
                t = loadp.tile([P, 4, D], F32R, tag="nat", name=f"nat_v{arg}")
                nc.sync.dma_start(
                    t, v_d[ts(arg, 4 * P), :].rearrange("(c p) d -> p c d", p=P))
                nat_v[arg] = t
            elif op == "dma_w2":
                nc.sync.dma_start(w2n, w2_d.rearrange("(c p) u -> p c u", p=P))
            elif op == "dma_x":
                nat_x = loadp.tile([P, 4, D], F32R, tag="nat", name="nat_x0")
                nc.sync.dma_start(
                    nat_x, q_d[ts(0, 4 * P), :].rearrange("(c p) d -> p c d", p=P))
            elif op == "tp":
                jc = arg
                g = jc // 4
                transpose_pair(nat_v[g][:, jc % 4, :], nat_v[g][:, jc % 4 + 1, :],
                               vT8, jc)
            elif op == "vn":
                emit_vn2(arg)
            elif op == "wt":
                emit_wt(w1n if arg == 1 else w2n, w1t if arg == 1 else w2t)
            elif op == "m12":
                emit_m12(arg)
            elif op == "xt":
                emit_xt(arg, nat_x[:, arg, :])
            elif op == "qmt":
                emit_qmt(0)
        # vN chunks 12..15 are deferred into phase 2


def _phase2(nc, tc, q_d, o_d, ident, m12_8, vT8, xT8, qmT8, vN, hilo, dr6,
            copy_f32r, w3_8):
    # One shared scratch-psum pool (4 x 512-f32 banks) serves score groups,
    # deferred X transposes, qmT and vN tail work; ctx keeps 2+2 banks.
    with tc.tile_pool(name="expp", bufs=2) as expp, \
         tc.tile_pool(name="loadp2", bufs=2) as loadp2, \
         tc.tile_pool(name="outp", bufs=4) as outp, \
         tc.tile_pool(name="wkps", bufs=4, space="PSUM") as wkps, \
         tc.tile_pool(name="caps", bufs=2, space="PSUM") as caps, \
         tc.tile_pool(name="cbps", bufs=2, space="PSUM") as cbps:
        ebias = outp.tile([P, 1], F32, name="ebias")
        nc.gpsimd.memset(ebias, -math.log(4.0))

        def emit_xt2(sc, nat):
            ps = wkps.tile([P, IB], F32, tag="wk")
            tpv = ps.bitcast(F32R).rearrange("p (c q) -> p c q", c=DC)
            for dc in range(DC):
                nc.tensor.transpose(tpv[:, dc, :], nat[:, ts(dc, P)], ident)
            hilo(xT8, [slice(None), ts(sc, P)], tpv, dve_hi=True)

        def emit_qmt2(ib):
            for ec in range(DC):
                ps = wkps.tile([P, IB], F32, tag="wk")
                dr6(ps, m12_8, [ts(ec, P)], xT8, [ts(ib, IB)])
                hilo(qmT8, [ec, ts(ib, IB)], ps, dve_hi=True)

        def emit_vn1(jc):
            ps = wkps.tile([P, IB], F32, tag="wk")
            dr6(ps, vT8, [ts(jc, P)], w3_8, [slice(None)])
            hilo(vN, [jc, slice(0, U)], ps, dve_hi=True)

        def emit_scores(ib):
            expB = expp.tile([P, SC, IB], F32R, name="expB")
            e8 = expp.tile([P, 2, SC, IB], F8, name="e8")
            for jc in range(SC):
                ps = wkps.tile([P, IB], F32, tag="wk")
                n = 0
                for pc in (0, 2):
                    for sh, mh in HL3:
                        n += 1
                        nc.tensor.matmul(
                            ps,
                            vT8[:, sh, pc:pc + 2, ts(jc, P)],
                            qmT8[:, mh, pc:pc + 2, ts(ib, IB)],
                            start=(n == 1), stop=(n == 6), perf_mode=DR)
                # bias -ln(4) keeps exp below the fp8e4 max (448) for the
                # largest scores; the ctx/den ratio cancels the 1/4 factor
                nc.scalar.activation(expB[:, jc, :], ps, EXP,
                                     bias=ebias, scale=SCALE / WSC)
                # fp8 hi/lo of exp for the DoubleRow ctx matmul; Pool does the
                # hi copy and DVE the residual so ACT stays exp-only
                nc.gpsimd.tensor_copy(e8[:, 0, jc, :], expB[:, jc, :])
                nc.vector.scalar_tensor_tensor(
                    e8[:, 1, jc, :], expB[:, jc, :], 1.0, e8[:, 0, jc, :],
                    op0=MUL, op1=SUB)
                if ib == 0 and jc in (5, 7, 9, 11):
                    emit_vn1(12 + (jc - 5) // 2)
            return e8

        def emit_ctx(e8, ib):
            for icc in range(ICC):
                i_glob = ib * ICC + icc
                psA = caps.tile([P, CA], F32, tag="ca")
                psB = cbps.tile([P, CB], F32, tag="cb")
                n = 0
                for jcp in range(0, SC, 2):
                    for sh, mh in HL3:
                        n += 1
                        st = e8[:, sh, jcp:jcp + 2, ts(icc, P)]
                        nc.tensor.matmul(psA, st,
                                         vN[:, mh, jcp:jcp + 2, 0:CA],
                                         start=(n == 1), stop=(n == 24),
                                         perf_mode=DR)
                        nc.tensor.matmul(psB, st,
                                         vN[:, mh, jcp:jcp + 2, CA:CA + CB],
                                         start=(n == 1), stop=(n == 24),
                                         perf_mode=DR)
                # den (= 16*sum_j exp) sits at psB col U-CA (=254)
                rec = outp.tile([P, 1], F32, tag="rec")
                nc.vector.reciprocal(rec, psB[:, U - CA:U - CA + 1])
                co = outp.tile([P, U], F32, tag="co")
                nc.vector.tensor_scalar_mul(co[:, 0:CA], psA, rec)
                nc.sync.dma_start(o_d[ts(i_glob, P), 0:CA], co[:, 0:CA])
                nc.scalar.mul(co[:, CA:U], psB[:, 0:U - CA], rec)
                nc.sync.dma_start(o_d[ts(i_glob, P), CA:U], co[:, CA:U])

        prev = None
        for ib in range(NIB):
            if ib < NIB - 1:
                nat_x = loadp2.tile([P, 4, D], F32R, tag="natx",
                                    name=f"nat_x{ib + 1}")
                nc.sync.dma_start(
                    nat_x,
                    q_d[ts(ib + 1, 4 * P), :].rearrange("(c p) d -> p c d", p=P))
            expB = emit_scores(ib)
            if ib < NIB - 1:
                for k in range(4):
                    emit_xt2(4 * (ib + 1) + k, nat_x[:, k, :])
                emit_qmt2(ib + 1)
            if prev is not None:
                emit_ctx(*prev)
            prev = (expB, ib)
        emit_ctx(*prev)


_PROGRAM = None


def _get_program():
    global _PROGRAM
    if _PROGRAM is None:
        nc = bacc.Bacc("TRN2", target_bir_lowering=False, debug=False,
                       num_devices=B)
        q_d = nc.dram_tensor("query", (S, D), F32R, kind="ExternalInput").ap()
        v_d = nc.dram_tensor("value", (S, D), F32R, kind="ExternalInput").ap()
        w1_d = nc.dram_tensor("W1", (D, U), F32R, kind="ExternalInput").ap()
        w2_d = nc.dram_tensor("W2", (D, U), F32R, kind="ExternalInput").ap()
        w3_d = nc.dram_tensor("W3", (D, U), F32R, kind="ExternalInput").ap()
        o_d = nc.dram_tensor("out", (S, U), F32, kind="ExternalOutput").ap()
        with tile.TileContext(nc) as tc:
            _emit(nc, tc, q_d, v_d, w1_d, w2_d, w3_d, o_d)
        nc.compile()
        _PROGRAM = nc
    return _PROGRAM


def kernel(**inputs) -> np.ndarray:
    query = np.ascontiguousarray(inputs["query"], dtype=np.float32)
    value = np.ascontiguousarray(inputs["value"], dtype=np.float32)
    W1 = np.ascontiguousarray(inputs["W1"], dtype=np.float32)
    W2 = np.ascontiguousarray(inputs["W2"], dtype=np.float32)
    W3 = np.ascontiguousarray(inputs["W3"], dtype=np.float32)
    assert query.shape == (B, S, D) and value.shape == (B, S, D)

    nc = _get_program()
    in_maps = [
        {"query": query[b], "value": value[b], "W1": W1, "W2": W2, "W3": W3}
        for b in range(B)
    ]
    res = run_bass_kernel_spmd(nc, in_maps, core_ids=list(range(B)))
    return np.stack([res.results[b]["out"] for b in range(B)], axis=0)


# revision 17
# speedup vs baseline: 1.0541x; 1.0541x over previous
"""Trainium2 Bass kernel for single-head attention (B=8, S=2048, D=U=512).

Sharding: data-parallel over batch - one batch element per NeuronCore (8 cores).

Math: score = X W1 (V W2)^T / sqrt(U) = X M V^T with M = W1 W2^T folded once
per core (saves one full projection). context = softmax(score) (V W3).

Dataflow per core:
  Phase 1 (streamed with the serial DMA queue: V0a,V0b,W3,V1,W1,V2,W2,V3,X0):
    - V chunks PE-transposed (f32r) and quantized to fp8e4 hi/lo (vT8).
    - W1,W2 PE-transposed; M = W1 W2^T on PE, scaled by 16, fp8 hi/lo (m12_8).
      W3 scaled by 16, fp8 hi/lo (w3_8).
    - vN[j,u] = 16*(V W3) via fp8 DoubleRow (3-term hi/lo cross products),
      f32r, with column 512 = 16.0 (fused softmax denominator).
    - X group 0 transposed/quantized (xT8); qmT(0) = 16*(M^T x^T) via
      DoubleRow, quantized hi/lo (qmT8).
  Phase 2 per i-block ib (pipelined):
    scores(ib): scoresT[j,i] = sum_e vT[e,j] qmT[e,i] via fp8 DoubleRow;
      exp on ACT -> expB (f32r). X group ib+1 transpose + qmT(ib+1) are
      interleaved here (PE) so their vector ops land in phase-2 ACT/DVE slack.
    ctx(ib-1): ctx[i,u] (+den col) = sum_j expB[j,i] vN[j,u], f32r matmuls in
      two psum groups (258+256 cols); out = ctx * recip(den) -> DMA.

fp8 DoubleRow (operands paired along a leading free dim of 2) runs at 0.5
cycles/output-col with 256-deep contraction per call - 4x f32r throughput;
the 3-call hi/lo scheme nets 2.67x at ~8-bit per-term accuracy. Measured
end-to-end rel err ~3e-3 (threshold 2e-2).
"""

import math
import os
import sys

for _p in ("/opt/trn_rl_repo", os.path.expanduser("~/.axon_site/_ro/trn_rl_repo")):
    if os.path.isdir(_p) and _p not in sys.path:
        sys.path.insert(0, _p)

import numpy as np

import concourse.bass as bass
import concourse.tile as tile
from concourse import bacc, mybir
from concourse.bass import ts
from concourse.bass_utils import run_bass_kernel_spmd
from concourse.masks import make_identity

F32 = mybir.dt.float32
F32R = mybir.dt.float32r
F8 = mybir.dt.float8e4
EXP = mybir.ActivationFunctionType.Exp
DR = mybir.MatmulPerfMode.DoubleRow
MUL = mybir.AluOpType.mult
SUB = mybir.AluOpType.subtract

P = 128          # partitions
B = 8            # batch (one element per core)
S = 2048         # sequence length
D = 512          # model dim
U = 512          # units
DC = D // P      # 4 chunks of the contraction dims
SC = S // P      # 16 s-chunks
IB = 512         # i-block (query positions per attention block)
NIB = S // IB    # 4
ICC = IB // P    # 4 i-chunks per block
SCALE = 1.0 / math.sqrt(float(U))
WSC = 16.0       # weight pre-scale so fp8 quantization stays in normal range
VNF = 520        # vN free width: 512 u-cols + col 512 = WSC (den) + pad
CA = 258         # ctx psum group A columns (u 0..257)
CB = 256         # ctx psum group B columns (u 258..511, den at 254, pad)

# DoubleRow hi/lo call list: (stationary half, moving half)
HL3 = ((0, 0), (0, 1), (1, 0))


def _emit(nc, tc, q_d, v_d, w1_d, w2_d, w3_d, o_d):
    with tc.tile_pool(name="const", bufs=1) as cp, \
         tc.tile_pool(name="persist", bufs=1) as pp:
        identf = cp.tile([P, P], F32, name="identf")
        make_identity(nc, identf)
        ident = cp.tile([P, P], F32R, name="ident")
        nc.vector.tensor_copy(ident, identf)

        m12_8 = pp.tile([P, 2, DC, D], F8, name="m12_8")
        w3_8 = pp.tile([P, 2, DC, U], F8, name="w3_8")
        vT8 = pp.tile([P, 2, DC, S], F8, name="vT8")
        xT8 = pp.tile([P, 2, DC, S], F8, name="xT8")
        qmT8 = pp.tile([P, 2, DC, S], F8, name="qmT8")
        vN = pp.tile([P, 2, SC, VNF], F8, name="vN")
        # den column = WSC in the hi plane (vN holds 16*v so the num/den
        # scales cancel); zero pad cols and the lo plane's den region.
        nc.gpsimd.memset(vN[:, 0, :, U:VNF], 0.0)
        nc.gpsimd.memset(vN[:, 1, :, U:VNF], 0.0)
        nc.gpsimd.memset(vN[:, 0, :, U:U + 1], WSC)

        _veng = [0]

        def hilo(dst8, hl_idx, src, scale=1.0, dve_hi=False):
            # dst8[...,0,...] = fp8(scale*src) on ACT (or DVE if dve_hi);
            # dst8[...,1,...] = fp8(scale*src - hi) on DVE.
            hi = dst8[tuple([slice(None), 0] + hl_idx)]
            lo = dst8[tuple([slice(None), 1] + hl_idx)]
            if dve_hi:
                nc.vector.tensor_copy(hi, src)
            elif scale == 1.0:
                nc.scalar.copy(hi, src)
            else:
                nc.scalar.mul(hi, src, scale)
            nc.vector.scalar_tensor_tensor(lo, src, scale, hi, op0=MUL, op1=SUB)

        def copy_f32r(dst, src):
            _veng[0] += 1
            if _veng[0] % 2:
                nc.scalar.copy(dst, src)
            else:
                nc.vector.tensor_copy(dst, src)

        def dr6(ps, stat8, s_idx, mov8, m_idx):
            # 6 DoubleRow calls: 2 pair-chunks x (hh, hl, lh)
            n = 0
            for pc in (0, 2):
                for sh, mh in HL3:
                    n += 1
                    nc.tensor.matmul(
                        ps,
                        stat8[tuple([slice(None), sh, slice(pc, pc + 2)] + s_idx)],
                        mov8[tuple([slice(None), mh, slice(pc, pc + 2)] + m_idx)],
                        start=(n == 1), stop=(n == 6), perf_mode=DR)

        _phase1(nc, tc, q_d, v_d, w1_d, w2_d, w3_d, ident,
                m12_8, w3_8, vT8, xT8, qmT8, vN, hilo, copy_f32r, dr6)
        _phase2(nc, tc, q_d, o_d, ident, m12_8, vT8, xT8, qmT8, vN, hilo, dr6,
                copy_f32r, w3_8)


def _phase1(nc, tc, q_d, v_d, w1_d, w2_d, w3_d, ident,
            m12_8, w3_8, vT8, xT8, qmT8, vN, hilo, copy_f32r, dr6):
    with tc.tile_pool(name="wtmp", bufs=1) as wp, \
         tc.tile_pool(name="loadp", bufs=3) as loadp, \
         tc.tile_pool(name="tps", bufs=2, space="PSUM") as tpsp, \
         tc.tile_pool(name="pjps", bufs=2, space="PSUM") as pjps:
        w1n = wp.tile([P, DC, U], F32R, name="w1n")
        w2n = wp.tile([P, DC, U], F32R, name="w2n")
        w3n = wp.tile([P, DC, U], F32R, name="w3n")
        w1t = wp.tile([P, DC, D], F32R, name="w1t")
        w2t = wp.tile([P, DC, D], F32R, name="w2t")

        def transpose_pair(nat0, nat1, dst8, jc):
            # two 128-row chunks -> one 2-bank psum tile (layout [c, g, q] so
            # (g q) merges to a contiguous 256-wide dim) -> one hi + one lo op
            tp = tpsp.tile([P, DC, 2, P], F32R, tag="tp")
            for g, nat in ((0, nat0), (1, nat1)):
                for dc in range(DC):
                    nc.tensor.transpose(tp[:, dc, g, :], nat[:, ts(dc, P)], ident)
            src = tp.rearrange("p c g q -> p c (g q)")
            hi = dst8[:, 0, :, jc * P:(jc + 2) * P]
            lo = dst8[:, 1, :, jc * P:(jc + 2) * P]
            nc.scalar.copy(hi, src)
            nc.vector.scalar_tensor_tensor(lo, src, 1.0, hi, op0=MUL, op1=SUB)

        def emit_wt(wn, wt):
            # wt[u%128, uc, d] = W[d, u] transposed blocks
            for ucp in (0, 2):
                tp = tpsp.tile([P, DC, 2, P], F32R, tag="tp")
                for g in (0, 1):
                    for dc in range(DC):
                        nc.tensor.transpose(tp[:, dc, g, :],
                                            wn[:, dc, ts(ucp + g, P)], ident)
                    copy_f32r(
                        wt[:, ucp + g, :].rearrange("p (k q) -> p k q", k=DC),
                        tp[:, :, g, :])

        def emit_m12(dcp):
            # M[d,e] = sum_u W1[d,u] W2[e,u], row-chunks (2dcp, 2dcp+1)
            ps = pjps.tile([P, 2, D], F32, tag="pj")
            for g in (0, 1):
                for uc in range(DC):
                    nc.tensor.matmul(ps[:, g, :], w1t[:, uc, ts(2 * dcp + g, P)],
                                     w2t[:, uc, :],
                                     start=(uc == 0), stop=(uc == DC - 1))
                hilo(m12_8, [2 * dcp + g, slice(None)], ps[:, g, :], scale=WSC)

        def emit_vn2(jc):
            # vN[j,u] = 16 * sum_d V[j,d] W3[d,u], two j-chunks at once
            ps = pjps.tile([P, 2, U], F32, tag="pj")
            dr6(ps[:, 0, :], vT8, [ts(jc, P)], w3_8, [slice(None)])
            dr6(ps[:, 1, :], vT8, [ts(jc + 1, P)], w3_8, [slice(None)])
            hilo(vN, [slice(jc, jc + 2), slice(0, U)], ps)

        def emit_xt(sc, nat):
            tp = tpsp.tile([P, DC, 2, P], F32R, tag="tp")
            for dc in range(DC):
                nc.tensor.transpose(tp[:, dc, 0, :], nat[:, ts(dc, P)], ident)
            hilo(xT8, [slice(None), ts(sc, P)], tp[:, :, 0, :])

        def emit_qmt(ib):
            # qmT[e,i] = 16 * sum_d M[d,e] xT[d,i] for i-block ib
            for ecp in (0, 2):
                ps = pjps.tile([P, 2, IB], F32, tag="pj")
                for g in (0, 1):
                    dr6(ps[:, g, :], m12_8, [ts(ecp + g, P)], xT8, [ts(ib, IB)])
                    hilo(qmT8, [ecp + g, ts(ib, IB)], ps[:, g, :])

        # DMA queue: V0a(chunk0), W3, V0b(1-3), V1, W1, V2, W2, V3, X0.
        # W3 right after the first chunk so vN matmuls can start early; the
        # vN stream then fills every later DMA-arrival stall.
        nat_v = [loadp.tile([P, 4, D], F32R, tag="nat", name=f"nat_v{g}")
                 for g in range(2)]
        for c in range(4):
            nc.sync.dma_start(
                nat_v[0][:, c:c + 1, :],
                v_d[ts(c, P), :].rearrange("(c p) d -> p c d", p=P))
        nc.sync.dma_start(w3n, w3_d.rearrange("(c p) u -> p c u", p=P))
        nc.sync.dma_start(nat_v[1],
                          v_d[ts(1, 4 * P), :].rearrange("(c p) d -> p c d", p=P))
        nc.sync.dma_start(w1n, w1_d.rearrange("(c p) u -> p c u", p=P))

        def tp_single(jc, nat):
            tp = tpsp.tile([P, DC, 2, P], F32R, tag="tp")
            for dc in range(DC):
                nc.tensor.transpose(tp[:, dc, 0, :], nat[:, ts(dc, P)], ident)
            hilo(vT8, [slice(None), ts(jc, P)], tp[:, :, 0, :])

        for c in range(4):
            tp_single(c, nat_v[0][:, c, :])
        hilo(w3_8, [slice(None), slice(None)], w3n, scale=WSC)
        nat_v.append(None)
        nat_v.append(None)
        sched = [
            ("vn", 0), ("vn", 2),
            ("dma_v", 2), ("dma_w2", None),
            ("tp", 4), ("vn", 4), ("tp", 6),
            ("wt", 1), ("vn", 6),
            ("dma_v", 3),
            ("tp", 8), ("vn", 8), ("tp", 10),
            ("wt", 2), ("vn", 10),
            ("dma_x", 0),
            ("m12", 0), ("tp", 12), ("m12", 1), ("tp", 14),
            ("xt", 0), ("xt", 1), ("xt", 2), ("xt", 3),
            ("qmt", 0),
        ]
        nat_x = None
        for op, arg in sched:
            if op == "dma_v":
                t = loadp.tile([P, 4, D], F32R, tag="nat", name=f"nat_v{arg}")
                nc.sync.dma_start(
                    t, v_d[ts(arg, 4 * P), :].rearrange("(c p) d -> p c d", p=P))
                nat_v[arg] = t
            elif op == "dma_w2":
                nc.sync.dma_start(w2n, w2_d.rearrange("(c p) u -> p c u", p=P))
            elif op == "dma_x":
                nat_x = loadp.tile([P, 4, D], F32R, tag="nat", name="nat_x0")
                nc.sync.dma_start(
                    nat_x, q_d[ts(0, 4 * P), :].rearrange("(c p) d -> p c d", p=P))
            elif op == "tp":
                jc = arg
                g = jc // 4
                transpose_pair(nat_v[g][:, jc % 4, :], nat_v[g][:, jc % 4 + 1, :],
                               vT8, jc)
            elif op == "vn":
                emit_vn2(arg)
            elif op == "wt":
                emit_wt(w1n if arg == 1 else w2n, w1t if arg == 1 else w2t)
            elif op == "m12":
                emit_m12(arg)
            elif op == "xt":
                emit_xt(arg, nat_x[:, arg, :])
            elif op == "qmt":
                emit_qmt(0)
        # vN chunks 12..15 are deferred into phase 2


def _phase2(nc, tc, q_d, o_d, ident, m12_8, vT8, xT8, qmT8, vN, hilo, dr6,
            copy_f32r, w3_8):
    # One shared scratch-psum pool (4 x 512-f32 banks) serves score groups,
    # deferred X transposes, qmT and vN tail work; ctx keeps 2+2 banks.
    with tc.tile_pool(name="expp", bufs=2) as expp, \
         tc.tile_pool(name="loadp2", bufs=2) as loadp2, \
         tc.tile_pool(name="outp", bufs=4) as outp, \
         tc.tile_pool(name="wkps", bufs=4, space="PSUM") as wkps, \
         tc.tile_pool(name="caps", bufs=2, space="PSUM") as caps, \
         tc.tile_pool(name="cbps", bufs=2, space="PSUM") as cbps:
        ebias = outp.tile([P, 1], F32, name="ebias")
        nc.gpsimd.memset(ebias, -math.log(4.0))

        def emit_xt2(sc, nat):
            ps = wkps.tile([P, IB], F32, tag="wk")
            tpv = ps.bitcast(F32R).rearrange("p (c q) -> p c q", c=DC)
            for dc in range(DC):
                nc.tensor.transpose(tpv[:, dc, :], nat[:, ts(dc, P)], ident)
            hilo(xT8, [slice(None), ts(sc, P)], tpv, dve_hi=True)

        def emit_qmt2(ib):
            for ec in range(DC):
                ps = wkps.tile([P, IB], F32, tag="wk")
                dr6(ps, m12_8, [ts(ec, P)], xT8, [ts(ib, IB)])
                hilo(qmT8, [ec, ts(ib, IB)], ps, dve_hi=True)

        def emit_vn1(jc):
            ps = wkps.tile([P, IB], F32, tag="wk")
            dr6(ps, vT8, [ts(jc, P)], w3_8, [slice(None)])
            hilo(vN, [jc, slice(0, U)], ps, dve_hi=True)

        def emit_scores(ib):
            expB = expp.tile([P, SC, IB], F32R, name="expB")
            e8 = expp.tile([P, 2, SC, IB], F8, name="e8")
            for jc in range(SC):
                ps = wkps.tile([P, IB], F32, tag="wk")
                n = 0
                for pc in (0, 2):
                    for sh, mh in HL3:
                        n += 1
                        nc.tensor.matmul(
                            ps,
                            vT8[:, sh, pc:pc + 2, ts(jc, P)],
                            qmT8[:, mh, pc:pc + 2, ts(ib, IB)],
                            start=(n == 1), stop=(n == 6), perf_mode=DR)
                # bias -ln(4) keeps exp below the fp8e4 max (448) for the
                # largest scores; the ctx/den ratio cancels the 1/4 factor
                nc.scalar.activation(expB[:, jc, :], ps, EXP,
                                     bias=ebias, scale=SCALE / WSC)
                # fp8 hi/lo of exp for the DoubleRow ctx matmul, two chunks
                # per op, hi/lo alternating between Pool and DVE so the two
                # chains run in parallel and ACT stays exp-only
                if jc % 2 == 1:
                    eB2 = expB[:, jc - 1:jc + 1, :]
                    hi2 = e8[:, 0, jc - 1:jc + 1, :]
                    lo2 = e8[:, 1, jc - 1:jc + 1, :]
                    if jc % 4 == 1:
                        nc.gpsimd.tensor_copy(hi2, eB2)
                        nc.vector.scalar_tensor_tensor(
                            lo2, eB2, 1.0, hi2, op0=MUL, op1=SUB)
                    else:
                        nc.vector.tensor_copy(hi2, eB2)
                        nc.gpsimd.tensor_tensor(lo2, eB2, hi2, op=SUB)
                if ib == 0 and jc in (5, 7, 9, 11):
                    emit_vn1(12 + (jc - 5) // 2)
            return e8

        def emit_ctx(e8, ib):
            for icc in range(ICC):
                i_glob = ib * ICC + icc
                psA = caps.tile([P, CA], F32, tag="ca")
                psB = cbps.tile([P, CB], F32, tag="cb")
                n = 0
                for jcp in range(0, SC, 2):
                    for sh, mh in HL3:
                        n += 1
                        st = e8[:, sh, jcp:jcp + 2, ts(icc, P)]
                        nc.tensor.matmul(psA, st,
                                         vN[:, mh, jcp:jcp + 2, 0:CA],
                                         start=(n == 1), stop=(n == 24),
                                         perf_mode=DR)
                        nc.tensor.matmul(psB, st,
                                         vN[:, mh, jcp:jcp + 2, CA:CA + CB],
                                         start=(n == 1), stop=(n == 24),
                                         perf_mode=DR)
                # den (= 16*sum_j exp) sits at psB col U-CA (=254)
                rec = outp.tile([P, 1], F32, tag="rec")
                nc.vector.reciprocal(rec, psB[:, U - CA:U - CA + 1])
                co = outp.tile([P, U], F32, tag="co")
                nc.vector.tensor_scalar_mul(co[:, 0:CA], psA, rec)
                nc.sync.dma_start(o_d[ts(i_glob, P), 0:CA], co[:, 0:CA])
                nc.scalar.mul(co[:, CA:U], psB[:, 0:U - CA], rec)
                nc.sync.dma_start(o_d[ts(i_glob, P), CA:U], co[:, CA:U])

        prev = None
        for ib in range(NIB):
            if ib < NIB - 1:
                nat_x = loadp2.tile([P, 4, D], F32R, tag="natx",
                                    name=f"nat_x{ib + 1}")
                nc.sync.dma_start(
                    nat_x,
                    q_d[ts(ib + 1, 4 * P), :].rearrange("(c p) d -> p c d", p=P))
            expB = emit_scores(ib)
            if ib < NIB - 1:
                for k in range(4):
                    emit_xt2(4 * (ib + 1) + k, nat_x[:, k, :])
                emit_qmt2(ib + 1)
            if prev is not None:
                emit_ctx(*prev)
            prev = (expB, ib)
        emit_ctx(*prev)


_PROGRAM = None


def _get_program():
    global _PROGRAM
    if _PROGRAM is None:
        nc = bacc.Bacc("TRN2", target_bir_lowering=False, debug=False,
                       num_devices=B)
        q_d = nc.dram_tensor("query", (S, D), F32R, kind="ExternalInput").ap()
        v_d = nc.dram_tensor("value", (S, D), F32R, kind="ExternalInput").ap()
        w1_d = nc.dram_tensor("W1", (D, U), F32R, kind="ExternalInput").ap()
        w2_d = nc.dram_tensor("W2", (D, U), F32R, kind="ExternalInput").ap()
        w3_d = nc.dram_tensor("W3", (D, U), F32R, kind="ExternalInput").ap()
        o_d = nc.dram_tensor("out", (S, U), F32, kind="ExternalOutput").ap()
        with tile.TileContext(nc) as tc:
            _emit(nc, tc, q_d, v_d, w1_d, w2_d, w3_d, o_d)
        nc.compile()
        _PROGRAM = nc
    return _PROGRAM


def kernel(**inputs) -> np.ndarray:
    query = np.ascontiguousarray(inputs["query"], dtype=np.float32)
    value = np.ascontiguousarray(inputs["value"], dtype=np.float32)
    W1 = np.ascontiguousarray(inputs["W1"], dtype=np.float32)
    W2 = np.ascontiguousarray(inputs["W2"], dtype=np.float32)
    W3 = np.ascontiguousarray(inputs["W3"], dtype=np.float32)
    assert query.shape == (B, S, D) and value.shape == (B, S, D)

    nc = _get_program()
    in_maps = [
        {"query": query[b], "value": value[b], "W1": W1, "W2": W2, "W3": W3}
        for b in range(B)
    ]
    res = run_bass_kernel_spmd(nc, in_maps, core_ids=list(range(B)))
    return np.stack([res.results[b]["out"] for b in range(B)], axis=0)
